# revision 1
# baseline (speedup 1.0000x reference)
"""Trainium2 Bass kernel for the recurrent-SE / depthwise-conv attention block.

Math per layer (faithful to the reference):
    pooled = mean(x, (2,3))                      # [B, C]
    ht, ct = cell(pooled, ht, ct)                # DSU cell, state [B, C]
    out_h, _ = cell(pooled, ht[0], ct[0])        # GLOBAL batch-0 state bcast
    x = x * (1 + out_h)[:, :, None, None] + dwconv3x3(x)

Sharding: data-parallel over batch, 8 samples/core. Because the second cell
reads the global sample-0 recurrent state, every core also carries a replica
of sample 0 (9 sample-plane-sets per core); the replica planes evolve like
any other sample so ht[0]/ct[0] stay locally available.

Per core:
  - x lives in SBUF in a zero-padded [30x30] per-(channel-block, sample)
    plane layout, channels on partitions (4 blocks of 128 channels), stored
    as f32r (the PE's reduced fp32) since it feeds fp32r conv matmuls.
  - dwconv3x3 runs on the TensorEngine as 9 accumulating matmuls per
    half-plane chunk with host-prebuilt diagonal tap matrices (fp32r).
  - The combine x*(1+out_h) + conv is one fused DVE scalar_tensor_tensor per
    half-plane whose accum_out also yields the pooled sums for the next
    layer (the 1/784 mean is folded into w_ih_l1 host-side).
  - The DSU cell runs on tiny fp32 matmuls; the l2 gate biases ride in an
    augmented ones-row of z1 so the gate sums need no separate bias ops.
    Cell matmul groups are interleaved between conv planes so the PE FIFO
    never idles on the serial cell chain.
"""

import numpy as np

import concourse.bacc as bacc
import concourse.bass as bass
import concourse.mybir as mybir
import concourse.tile as tile
from concourse.bass_utils import run_bass_kernel_spmd

F32 = mybir.dt.float32
F32R = mybir.dt.float32r
ALU = mybir.AluOpType
ACTF = mybir.ActivationFunctionType
AX = mybir.AxisListType

N_CORES = 8
B_FULL, C, H, W = 64, 512, 28, 28
B_SH = B_FULL // N_CORES           # 8 shard samples per core
BL = B_SH + 1                      # +1 replica of global sample 0
CB = C // 128                      # 4 channel blocks
HW = H * W                         # 784
PR, PC = H + 2, W + 2              # padded plane 30 x 30
PLANE = PR * PC                    # 900
NPLANE = CB * BL                   # 36 planes per core
HALF = H // 2                      # 14 rows per half-plane chunk
NCHUNK = HALF * W                  # 392 columns per conv matmul
G3 = 3 * NPLANE                    # 108 gate columns

# packed single-bank cell PSUM layout (columns of cellps)
ZC1 = 2 * BL + 1                   # z1 pre-activations [33p, 19]
GI0, GI1 = ZC1, ZC1 + G3           # g_i (+bias) 12 x 9
GH0, GH1 = GI1, GI1 + G3           # g_h 12 x 9
G20, G21 = GH1, GH1 + 12           # g_h2 (batch-0 bcast) 12 x 1


def build_program(num_layers: int = 4, iters: int = 1,
                  skip_cells: bool = False, skip_io: bool = False):
    nc = bacc.Bacc("TRN2", target_bir_lowering=False, debug=False,
                   num_devices=N_CORES)

    x_d = nc.dram_tensor("x", [BL, C, H, W], F32, kind="ExternalInput").ap()
    diag_d = nc.dram_tensor("diag", [CB * 9 * 128, 128], F32R,
                            kind="ExternalInput").ap()
    wih1t_d = nc.dram_tensor("wih1t", [C, 32], F32, kind="ExternalInput").ap()
    whh1t_d = nc.dram_tensor("whh1t", [C, 32], F32, kind="ExternalInput").ap()
    wih2t_d = nc.dram_tensor("wih2t", [33, 3 * C], F32, kind="ExternalInput").ap()
    whh2t_d = nc.dram_tensor("whh2t", [33, 3 * C], F32, kind="ExternalInput").ap()
    b1_d = nc.dram_tensor("b1", [32, 2], F32, kind="ExternalInput").ap()
    y_d = nc.dram_tensor("y", [B_SH, C, H, W], F32, kind="ExternalOutput").ap()

    with tile.TileContext(nc) as tc:
        with (
            tc.tile_pool(name="persist", bufs=1) as pp,
            tc.tile_pool(name="stagep", bufs=3) as sp,
            tc.tile_pool(name="spillp", bufs=12) as spl,
            tc.tile_pool(name="convps", bufs=7, space="PSUM") as cvp,
            tc.tile_pool(name="cellps", bufs=1, space="PSUM") as clp,
        ):
            # +PC slack so the last plane's shifted window slice stays in range
            xpad = pp.tile([128, NPLANE * PLANE + PC], F32R, tag="xpad")
            diag_sb = pp.tile([128, CB * 9 * 128], F32R, tag="diag")
            wih1t_sb = pp.tile([128, CB * 32], F32, tag="wih1t")
            whh1t_sb = pp.tile([128, CB * 32], F32, tag="whh1t")
            wih2t_sb = pp.tile([33, 3 * C], F32, tag="wih2t")
            whh2t_sb = pp.tile([33, 3 * C], F32, tag="whh2t")
            b1_sb = pp.tile([32, 2], F32, tag="b1")

            pooled = pp.tile([128, NPLANE], F32, tag="pooled")
            poolacc = pp.tile([128, NPLANE * 2], F32, tag="poolacc")
            ht = pp.tile([128, NPLANE], F32, tag="ht")
            ct = pp.tile([128, NPLANE], F32, tag="ct")
            z1 = pp.tile([33, 2 * BL + 1], F32, tag="z1")
            gates = pp.tile([128, G3], F32, tag="gates")
            sgi = pp.tile([128, G3], F32, tag="sgi")
            tmt = pp.tile([128, G3], F32, tag="tmt")
            gi_sb = pp.tile([128, G3], F32, tag="gi_sb")
            s_sb = pp.tile([128, NPLANE], F32, tag="s_sb")

            cellps = clp.tile([128, G21], F32, tag="cellps")

            # constants in once
            nc.sync.dma_start(
                diag_sb[:, :].rearrange("p (blk m) -> p blk m", m=128),
                diag_d.rearrange("(blk k) m -> k blk m", k=128))
            nc.sync.dma_start(
                wih1t_sb[:, :].rearrange("p (cb m) -> p cb m", m=32),
                wih1t_d.rearrange("(cb k) m -> k cb m", k=128))
            nc.sync.dma_start(
                whh1t_sb[:, :].rearrange("p (cb m) -> p cb m", m=32),
                whh1t_d.rearrange("(cb k) m -> k cb m", k=128))
            nc.sync.dma_start(wih2t_sb[:, :], wih2t_d)
            nc.sync.dma_start(whh2t_sb[:, :], whh2t_d)
            nc.sync.dma_start(b1_sb[:, :], b1_d)
            # ones row for the augmented-bias matmuls
            nc.vector.memset(z1[32:33, :], 1.0)
            # zero the padded buffer once; interiors are overwritten each
            # layer, borders stay zero forever. (memset can't write f32r, so
            # zero a small f32 tile and fan it out through DVE copies)
            z0 = sp.tile([128, PLANE + PC], F32, tag="stage", name="z0")
            nc.vector.memset(z0[:, :], 0.0)
            for pl in range(NPLANE):
                nc.vector.tensor_copy(
                    xpad[:, pl * PLANE:(pl + 1) * PLANE], z0[:, 0:PLANE])
            nc.vector.tensor_copy(
                xpad[:, NPLANE * PLANE:NPLANE * PLANE + PC], z0[:, 0:PC])

            flat = xpad[:, :]

            def intr(pl, r0, nr):
                """interior window [128, nr, 28] of plane pl at row r0."""
                off = pl * PLANE + (r0 + 1) * PC + 1
                return flat[:, off:off + nr * PC].rearrange(
                    "p (r w) -> p r w", w=PC)[:, :, 0:W]

            def shifted(pl, r0, dy, dx):
                off = pl * PLANE + (r0 + 1 + dy) * PC + 1 + dx
                return flat[:, off:off + HALF * PC].rearrange(
                    "p (r w) -> p r w", w=PC)[:, :, 0:W]

            def cell_stage(st, first_layer):
                """The DSU cell in 5 stages so PE work can interleave with
                conv planes.  pooled, ht, ct -> new ht, ct; s = 1+out_h."""
                if st == 0:  # z1 pre-activations (PE)
                    for cb in range(CB):
                        nc.tensor.matmul(
                            cellps[0:32, 0:BL],
                            wih1t_sb[:, cb * 32:(cb + 1) * 32],
                            pooled[:, cb * BL:(cb + 1) * BL],
                            start=(cb == 0), stop=(cb == CB - 1))
                    if first_layer:
                        # ht == 0 -> hh path contributes relu(b_hh1)
                        nc.vector.memset(cellps[0:32, BL:2 * BL], 0.0)
                    else:
                        for cb in range(CB):
                            nc.tensor.matmul(
                                cellps[0:32, BL:2 * BL],
                                whh1t_sb[:, cb * 32:(cb + 1) * 32],
                                ht[:, cb * BL:(cb + 1) * BL],
                                start=(cb == 0), stop=(cb == CB - 1))
                elif st == 1:  # relu, then gate matmuls (PE bulk)
                    nc.scalar.activation(z1[0:32, 0:BL], cellps[0:32, 0:BL],
                                         ACTF.Relu, bias=b1_sb[:, 0:1])
                    nc.scalar.activation(z1[0:32, BL:2 * BL],
                                         cellps[0:32, BL:2 * BL],
                                         ACTF.Relu, bias=b1_sb[:, 1:2])
                    for g in range(3):
                        for cb in range(CB):
                            co = (g * CB + cb) * BL
                            wsl = slice(g * C + cb * 128,
                                        g * C + (cb + 1) * 128)
                            nc.tensor.matmul(
                                cellps[:, GI0 + co:GI0 + co + BL],
                                wih2t_sb[:, wsl], z1[:, 0:BL],
                                start=True, stop=True)
                            nc.tensor.matmul(
                                cellps[:, GH0 + co:GH0 + co + BL],
                                whh2t_sb[:, wsl], z1[:, BL:2 * BL],
                                start=True, stop=True)
                elif st == 2:  # cell 1 state update (DVE/ACT)
                    nc.vector.tensor_copy(gi_sb[:, :], cellps[:, GI0:GI1])
                    nc.vector.tensor_tensor(gates[:, :], gi_sb[:, :],
                                            cellps[:, GH0:GH1], ALU.add)
                    nc.scalar.activation(sgi[:, 0:NPLANE], gates[:, 0:NPLANE],
                                         ACTF.Sigmoid)
                    nc.scalar.activation(sgi[:, NPLANE:2 * NPLANE],
                                         gates[:, NPLANE:2 * NPLANE],
                                         ACTF.Sigmoid)
                    nc.scalar.activation(sgi[:, 2 * NPLANE:G3],
                                         gates[:, 2 * NPLANE:G3], ACTF.Tanh)
                    nc.vector.tensor_tensor(tmt[:, 0:NPLANE], sgi[:, 0:NPLANE],
                                            sgi[:, 2 * NPLANE:G3], ALU.mult)
                    if first_layer:
                        nc.vector.tensor_copy(ct[:, :], tmt[:, 0:NPLANE])
                    else:
                        nc.vector.tensor_tensor(
                            tmt[:, NPLANE:2 * NPLANE],
                            sgi[:, NPLANE:2 * NPLANE], ct[:, :], ALU.mult)
                        nc.vector.tensor_tensor(ct[:, :], tmt[:, 0:NPLANE],
                                                tmt[:, NPLANE:2 * NPLANE],
                                                ALU.add)
                    nc.scalar.activation(ht[:, :], ct[:, :], ACTF.Sigmoid)
                elif st == 3:  # cell 2 hh path from ht[0] (PE)
                    for cb in range(CB):
                        nc.tensor.matmul(
                            cellps[0:32, 2 * BL:2 * BL + 1],
                            whh1t_sb[:, cb * 32:(cb + 1) * 32],
                            ht[:, cb * BL:cb * BL + 1],
                            start=(cb == 0), stop=(cb == CB - 1))
                    nc.scalar.activation(z1[0:32, 2 * BL:2 * BL + 1],
                                         cellps[0:32, 2 * BL:2 * BL + 1],
                                         ACTF.Relu, bias=b1_sb[:, 1:2])
                    for g in range(3):
                        for cb in range(CB):
                            j = G20 + g * CB + cb
                            nc.tensor.matmul(
                                cellps[:, j:j + 1],
                                whh2t_sb[:, g * C + cb * 128:
                                         g * C + (cb + 1) * 128],
                                z1[:, 2 * BL:2 * BL + 1],
                                start=True, stop=True)
                else:  # st == 4: cell 2 -> s = 1 + out_h (DVE/ACT)
                    # gates2 = (g_i + bias) + g_h2 broadcast over batch
                    nc.vector.tensor_tensor(
                        gates[:, :].rearrange("p (j b) -> p j b", b=BL),
                        gi_sb[:, :].rearrange("p (j b) -> p j b", b=BL),
                        cellps[:, G20:G21].unsqueeze(-1).broadcast_to(
                            [128, 12, BL]),
                        ALU.add)
                    nc.scalar.activation(sgi[:, 0:NPLANE], gates[:, 0:NPLANE],
                                         ACTF.Sigmoid)
                    nc.scalar.activation(sgi[:, NPLANE:2 * NPLANE],
                                         gates[:, NPLANE:2 * NPLANE],
                                         ACTF.Sigmoid)
                    nc.scalar.activation(sgi[:, 2 * NPLANE:G3],
                                         gates[:, 2 * NPLANE:G3], ACTF.Tanh)
                    nc.vector.tensor_tensor(tmt[:, 0:NPLANE], sgi[:, 0:NPLANE],
                                            sgi[:, 2 * NPLANE:G3], ALU.mult)
                    # ncx2 = sig(f2)*ct[0] + sig(i2)*tanh(c2)
                    for cb in range(CB):
                        bs = cb * BL
                        nc.vector.scalar_tensor_tensor(
                            tmt[:, NPLANE + bs:NPLANE + bs + BL],
                            sgi[:, NPLANE + bs:NPLANE + bs + BL],
                            ct[:, bs:bs + 1], tmt[:, bs:bs + BL],
                            ALU.mult, ALU.add)
                    nc.scalar.activation(tmt[:, 2 * NPLANE:G3],
                                         tmt[:, NPLANE:2 * NPLANE],
                                         ACTF.Sigmoid)
                    nc.vector.tensor_scalar(s_sb[:, :], tmt[:, 2 * NPLANE:G3],
                                            1.0, None, ALU.add)

            def emit_conv(pl):
                """18 conv matmuls of one plane -> 2 SBUF spill tiles.
                The ACT eviction isn't gated on s, so PSUM banks recycle at
                PE pace and the PE never stalls on the combine chain."""
                cb = pl // BL
                scs = []
                for hf in range(2):
                    r0 = hf * HALF
                    ps = cvp.tile([128, NCHUNK], F32, tag="cps", name="cps")
                    for t in range(9):
                        dy, dx = t // 3 - 1, t % 3 - 1
                        nc.tensor.matmul(
                            ps[:, :],
                            diag_sb[:, (cb * 9 + t) * 128:
                                    (cb * 9 + t + 1) * 128],
                            shifted(pl, r0, dy, dx),
                            start=(t == 0), stop=(t == 8))
                    sc = spl.tile([128, NCHUNK], F32, tag="spill", name="sc")
                    nc.scalar.copy(sc[:, :], ps[:, :])
                    scs.append(sc)
                return scs

            def emit_combine(pl, scs, last):
                cb = pl // BL
                ost = (sp.tile([128, HW], F32, tag="stage", name="ost")
                       if last else None)
                for hf in range(2):
                    r0 = hf * HALF
                    sc = scs[hf]
                    src = intr(pl, r0, HALF)
                    if last:
                        dst = ost[:, r0 * W:(r0 + HALF) * W].rearrange(
                            "p (r w) -> p r w", w=W)
                        acc = None
                    else:
                        dst = src
                        acc = poolacc[:, pl * 2 + hf:pl * 2 + hf + 1]
                    nc.vector.scalar_tensor_tensor(
                        dst, src, s_sb[:, pl:pl + 1],
                        sc[:, :].rearrange("p (r w) -> p r w", w=W),
                        ALU.mult, ALU.add, accum_out=acc)
                if last:
                    b = pl % BL
                    nc.sync.dma_start(
                        y_d[b - 1, cb * 128:(cb + 1) * 128, :, :],
                        ost[:, :].rearrange("p (h w) -> p h w", w=W))

            def emit_plane(pl, last):
                emit_combine(pl, emit_conv(pl), last)

            def emit_input():
                # DRAM -> stage (paired planes, both HWDGE rings) -> padded
                # layout; also the layer-1 pooled sums
                for cb in range(CB):
                    groups = [(0, 2), (2, 2), (4, 2), (6, 2), (8, 1)]
                    for gi, (b0, nb) in enumerate(groups):
                        stage = sp.tile([128, 2 * HW], F32, tag="stage",
                                        name="stage")
                        eng = nc.sync if gi % 2 == 0 else nc.scalar
                        eng.dma_start(
                            stage[:, 0:nb * HW].rearrange(
                                "p (b hw) -> p b hw", hw=HW),
                            x_d[b0:b0 + nb, cb * 128:(cb + 1) * 128, :, :]
                            .rearrange("b c h w -> c b (h w)"))
                        for k in range(nb):
                            pl = cb * BL + b0 + k
                            seg = stage[:, k * HW:(k + 1) * HW]
                            nc.vector.tensor_reduce(
                                pooled[:, pl:pl + 1], seg, AX.X, ALU.add)
                            # DVE, not ACT: the f32->f32r conversion on a
                            # strided ACT copy measures ~3x slower
                            nc.vector.tensor_copy(
                                intr(pl, 0, H),
                                seg.rearrange("p (h w) -> p h w", w=W))

            if skip_io:
                emit_input()
            if skip_cells:
                nc.vector.memset(s_sb[:, :], 1.5)

            def emit_body():
                if not skip_io:
                    emit_input()
                for layer in range(num_layers):
                    last = layer == num_layers - 1
                    if layer > 0 and not skip_cells:
                        # pooled = half0 + half1 of the combine accumulators
                        nc.vector.tensor_tensor(
                            pooled[:, :],
                            poolacc[:, 0:2 * NPLANE:2],
                            poolacc[:, 1:2 * NPLANE:2],
                            ALU.add)
                    planes = [cb * BL + b for cb in range(CB)
                              for b in range(BL)
                              if not (last and b == 0)]
                    if skip_cells:
                        for pl in planes:
                            emit_plane(pl, last)
                        continue
                    # interleave the serial cell chain with the first conv
                    # planes so the PE FIFO doesn't idle on it; their
                    # combines wait until s is available (emitted after
                    # stage 4 so the dependency binds to THIS layer's s)
                    early = planes[0:3]
                    held = []
                    for st in range(5):
                        cell_stage(st, layer == 0)
                        if st < len(early):
                            held.append((early[st], emit_conv(early[st])))
                    for pl, pss in held:
                        emit_combine(pl, pss, last)
                    for pl in planes[len(early):]:
                        emit_plane(pl, last)

            if iters == 1:
                emit_body()
            else:
                with tc.For_i(0, iters, 1):
                    emit_body()

    nc.compile()
    return nc


def prep_inputs(x, w_ih_l1, b_ih_l1, w_ih_l2, b_ih_l2,
                w_hh_l1, b_hh_l1, w_hh_l2, b_hh_l2, dw_kernel):
    """Host-side prep: per-core input maps (weights replicated)."""
    x = np.ascontiguousarray(np.asarray(x, dtype=np.float32))
    diag = np.zeros((CB, 9, 128, 128), np.float32)
    dw = np.asarray(dw_kernel, np.float32).reshape(C, 9)
    idx = np.arange(128)
    for cb in range(CB):
        for t in range(9):
            diag[cb, t, idx, idx] = dw[cb * 128:(cb + 1) * 128, t]
    # l2 weights pre-transposed with the summed gate bias as an extra row
    # (pairs with the ones-row of z1)
    wih2t = np.concatenate(
        [np.asarray(w_ih_l2, np.float32).T,
         (np.asarray(b_ih_l2, np.float32)
          + np.asarray(b_hh_l2, np.float32))[None, :]], axis=0)
    whh2t = np.concatenate(
        [np.asarray(w_hh_l2, np.float32).T,
         np.zeros((1, 3 * C), np.float32)], axis=0)
    common = {
        "diag": diag.reshape(CB * 9 * 128, 128),
        "wih1t": np.ascontiguousarray(
            (np.asarray(w_ih_l1, np.float32) / HW).T),
        "whh1t": np.ascontiguousarray(np.asarray(w_hh_l1, np.float32).T),
        "wih2t": np.ascontiguousarray(wih2t),
        "whh2t": np.ascontiguousarray(whh2t),
        "b1": np.ascontiguousarray(np.stack(
            [np.asarray(b_ih_l1, np.float32),
             np.asarray(b_hh_l1, np.float32)], axis=1)),
    }
    return [dict(common, x=np.ascontiguousarray(np.concatenate(
        [x[0:1], x[i * B_SH:(i + 1) * B_SH]], axis=0)))
        for i in range(N_CORES)]


_cache = {}


def kernel(**inputs) -> np.ndarray:
    num_layers = int(inputs["num_layers"])
    if num_layers not in _cache:
        _cache[num_layers] = build_program(num_layers=num_layers, iters=1)
    nc = _cache[num_layers]
    in_maps = prep_inputs(
        inputs["x"], inputs["w_ih_l1"], inputs["b_ih_l1"], inputs["w_ih_l2"],
        inputs["b_ih_l2"], inputs["w_hh_l1"], inputs["b_hh_l1"],
        inputs["w_hh_l2"], inputs["b_hh_l2"], inputs["dw_kernel"])
    res = run_bass_kernel_spmd(nc, in_maps, list(range(N_CORES)))
    return np.concatenate([res.results[i]["y"] for i in range(N_CORES)],
                          axis=0).astype(np.float32)



# revision 21
# speedup vs baseline: 12.1124x; 12.1124x over previous
"""Trainium2 Bass kernel for the recurrent-SE / depthwise-conv attention block.

Math per layer (faithful to the reference):
    pooled = mean(x, (2,3))                      # [B, C]
    ht, ct = cell(pooled, ht, ct)                # DSU cell, state [B, C]
    out_h, _ = cell(pooled, ht[0], ct[0])        # GLOBAL batch-0 state bcast
    x = x * (1 + out_h)[:, :, None, None] + dwconv3x3(x)

Sharding: data-parallel over batch, 8 samples/core. Because the second cell
reads the global sample-0 recurrent state, every core also carries a replica
of sample 0 (9 sample-plane-sets per core); the replica planes evolve like
any other sample so ht[0]/ct[0] stay locally available.

Per core, x lives in SBUF as bf16 zero-padded [30 x 32] planes (channels on
partitions, 4 blocks of 128 channels). Each plane is one of two types whose
ratio balances PE vs DVE load:

  type C ("PE-complete"): all 9 taps are accumulating diag matmuls on the
    TensorEngine, with the combine's s-multiply FOLDED into the center tap:
    its stationary is diag(w_center + s_plane), built per plane per layer by
    one tiny DVE tensor_scalar (identity * (w00+s)). PSUM then holds the
    complete layer output y = s*x + conv(x); ACT evicts it to bf16 x (or the
    f32 output staging tile on the last layer) with accum_out emitting the
    pooled sum. These planes cost DVE nothing but one 94ns diag build.

  type B ("split"): the 6 dx!=0 taps on PE, evicted by ACT to a bf16 spill;
    the 3 dx=0 taps run on DVE as tensor_scalar products (4x mode) +
    tensor_tensor adds (2x mode); the final combine y = s*x + acc is one
    scalar_tensor_tensor with accum_out -> pooled.

The DSU cell runs on tiny fp32 matmuls; the l2 gate biases ride in an
augmented ones-row of z1. Each layer emits two conv planes BEFORE the cell
chain so the PE never idles waiting for the previous layer's last pooled
sums, and the s-gated work (diag builds, type-C planes, combines) is emitted
after cell stage 4 so it never head-blocks an engine queue.
"""

import numpy as np
import ml_dtypes

import concourse.bacc as bacc
import concourse.bass as bass
import concourse.mybir as mybir
import concourse.tile as tile
from concourse.bass_utils import run_bass_kernel_spmd

F32 = mybir.dt.float32
BF16 = mybir.dt.bfloat16
ALU = mybir.AluOpType
ACTF = mybir.ActivationFunctionType
AX = mybir.AxisListType

N_CORES = 8
B_FULL, C, H, W = 64, 512, 28, 28
B_SH = B_FULL // N_CORES           # 8 shard samples per core
BL = B_SH + 1                      # +1 replica of global sample 0
CB = C // 128                      # 4 channel blocks
HW = H * W                         # 784
PR, PC = H + 2, 32                 # padded plane 30 rows x 32 cols
PLANE = PR * PC                    # 960
COL0 = 2                           # interior column start (even => aligned)
NPLANE = CB * BL                   # 36 planes per core
HALF = H // 2                      # 14 rows per half-plane conv chunk
NCHUNK = HALF * W                  # 392 columns per conv matmul
G3 = 3 * NPLANE                    # 108 gate columns

ALL_TAPS = [(dy, dx) for dy in (-1, 0, 1) for dx in (-1, 0, 1)]   # 9
PE_TAPS8 = [t for t in ALL_TAPS if t != (0, 0)]                   # 8 (type C)
PE_TAPS6 = [(dy, dx) for dy in (-1, 0, 1) for dx in (-1, 1)]      # 6 (type B)
# type-B DVE taps: only dy=+-1; the (0,0) tap rides the final combine's
# per-partition scalar as (w00 + s), shared with the type-C diag fold
DVE_TAPS = [(-1, 0), (1, 0)]

# packed single-bank cell PSUM layout (columns of cellps)
ZC1 = 2 * BL + 1                   # z1 pre-activations [33p, 19]
GI0, GI1 = ZC1, ZC1 + G3           # g_i (+bias) 12 x 9
GH0, GH1 = GI1, GI1 + G3           # g_h 12 x 9
G20, G21 = GH1, GH1 + 12           # g_h2 (batch-0 bcast) 12 x 1


def build_program(num_layers: int = 4, iters: int = 1, n_c: int = 9,
                  skip_cells: bool = False, skip_io: bool = False):
    nc = bacc.Bacc("TRN2", target_bir_lowering=False, debug=False,
                   num_devices=N_CORES)

    x_d = nc.dram_tensor("x", [BL, C, H, W], F32, kind="ExternalInput").ap()
    diag_d = nc.dram_tensor("diag", [CB * 9 * 128, 128], BF16,
                            kind="ExternalInput").ap()
    ident_d = nc.dram_tensor("ident", [128, 128], BF16,
                             kind="ExternalInput").ap()
    dwv_d = nc.dram_tensor("dwv", [128, CB * 2], F32, kind="ExternalInput").ap()
    dw00r_d = nc.dram_tensor("dw00r", [128, NPLANE], F32,
                             kind="ExternalInput").ap()
    wih1t_d = nc.dram_tensor("wih1t", [C, 32], F32, kind="ExternalInput").ap()
    whh1t_d = nc.dram_tensor("whh1t", [C, 32], F32, kind="ExternalInput").ap()
    wih2t_d = nc.dram_tensor("wih2t", [33, 3 * C], F32, kind="ExternalInput").ap()
    whh2t_d = nc.dram_tensor("whh2t", [33, 3 * C], F32, kind="ExternalInput").ap()
    b1_d = nc.dram_tensor("b1", [32, 2], F32, kind="ExternalInput").ap()
    y_d = nc.dram_tensor("y", [B_SH, C, H, W], F32, kind="ExternalOutput").ap()

    with tile.TileContext(nc) as tc:
        with (
            tc.tile_pool(name="persist", bufs=1) as pp,
            tc.tile_pool(name="stagep", bufs=3) as sp,
            tc.tile_pool(name="spillp", bufs=4) as spl,
            tc.tile_pool(name="accp", bufs=33) as acp,
            tc.tile_pool(name="tpp", bufs=3) as tpp,
            tc.tile_pool(name="d00p", bufs=12) as d0p,
            tc.tile_pool(name="ostp", bufs=4) as osp,
            tc.tile_pool(name="convps", bufs=3, space="PSUM") as cvp,
            tc.tile_pool(name="cellps", bufs=1, space="PSUM") as clp,
        ):
            # +PC slack so the last plane's shifted window slice stays in range
            xpad = pp.tile([128, NPLANE * PLANE + PC], BF16, tag="xpad")
            diag_sb = pp.tile([128, CB * 9 * 128], BF16, tag="diag")
            ident_sb = pp.tile([128, 128], BF16, tag="ident")
            dwv_sb = pp.tile([128, CB * 2], F32, tag="dwv")
            dw00r_sb = pp.tile([128, NPLANE], F32, tag="dw00r")
            wih1t_sb = pp.tile([128, CB * 32], F32, tag="wih1t")
            whh1t_sb = pp.tile([128, CB * 32], F32, tag="whh1t")
            wih2t_sb = pp.tile([33, 3 * C], F32, tag="wih2t")
            whh2t_sb = pp.tile([33, 3 * C], F32, tag="whh2t")
            b1_sb = pp.tile([32, 2], F32, tag="b1")

            pooled = pp.tile([128, NPLANE], F32, tag="pooled")
            ht = pp.tile([128, NPLANE], F32, tag="ht")
            ct = pp.tile([128, NPLANE], F32, tag="ct")
            z1 = pp.tile([33, 2 * BL + 1], F32, tag="z1")
            gates = pp.tile([128, G3], F32, tag="gates")
            sgi = pp.tile([128, G3], F32, tag="sgi")
            tmt = pp.tile([128, G3], F32, tag="tmt")
            gi_sb = pp.tile([128, G3], F32, tag="gi_sb")
            s_sb = pp.tile([128, NPLANE], F32, tag="s_sb")
            w00s = pp.tile([128, NPLANE], F32, tag="w00s")

            cellps = clp.tile([128, G21], F32, tag="cellps")

            # constants in once
            nc.sync.dma_start(
                diag_sb[:, :].rearrange("p (blk m) -> p blk m", m=128),
                diag_d.rearrange("(blk k) m -> k blk m", k=128))
            nc.sync.dma_start(ident_sb[:, :], ident_d)
            nc.sync.dma_start(dwv_sb[:, :], dwv_d)
            nc.sync.dma_start(dw00r_sb[:, :], dw00r_d)
            nc.sync.dma_start(
                wih1t_sb[:, :].rearrange("p (cb m) -> p cb m", m=32),
                wih1t_d.rearrange("(cb k) m -> k cb m", k=128))
            nc.sync.dma_start(
                whh1t_sb[:, :].rearrange("p (cb m) -> p cb m", m=32),
                whh1t_d.rearrange("(cb k) m -> k cb m", k=128))
            nc.sync.dma_start(wih2t_sb[:, :], wih2t_d)
            nc.sync.dma_start(whh2t_sb[:, :], whh2t_d)
            nc.sync.dma_start(b1_sb[:, :], b1_d)
            # ones row for the augmented-bias matmuls
            nc.vector.memset(z1[32:33, :], 1.0)
            # zero the padded buffer once; interiors are overwritten each
            # layer, borders stay zero forever.
            nc.vector.memset(xpad[:, :], 0.0)

            flat = xpad[:, :]

            def intr(pl):
                """interior window [128, 28, 28] of plane pl (bf16)."""
                off = pl * PLANE + PC + COL0
                return flat[:, off:off + H * PC].rearrange(
                    "p (r w) -> p r w", w=PC)[:, :, 0:W]

            def tap(pl, dy, dx, r0=0, nr=H):
                off = pl * PLANE + (r0 + 1 + dy) * PC + COL0 + dx
                return flat[:, off:off + nr * PC].rearrange(
                    "p (r w) -> p r w", w=PC)[:, :, 0:W]

            def cell_stage(st, first_layer):
                """The DSU cell in 5 stages so PE work can interleave with
                conv planes.  pooled, ht, ct -> new ht, ct; s = 1+out_h."""
                if st == 0:  # z1 pre-activations (PE)
                    for cb in range(CB):
                        nc.tensor.matmul(
                            cellps[0:32, 0:BL],
                            wih1t_sb[:, cb * 32:(cb + 1) * 32],
                            pooled[:, cb * BL:(cb + 1) * BL],
                            start=(cb == 0), stop=(cb == CB - 1))
                    if first_layer:
                        # ht == 0 -> hh path contributes relu(b_hh1)
                        nc.vector.memset(cellps[0:32, BL:2 * BL], 0.0)
                    else:
                        for cb in range(CB):
                            nc.tensor.matmul(
                                cellps[0:32, BL:2 * BL],
                                whh1t_sb[:, cb * 32:(cb + 1) * 32],
                                ht[:, cb * BL:(cb + 1) * BL],
                                start=(cb == 0), stop=(cb == CB - 1))
                elif st == 1:  # relu, then gate matmuls (PE bulk)
                    nc.scalar.activation(z1[0:32, 0:BL], cellps[0:32, 0:BL],
                                         ACTF.Relu, bias=b1_sb[:, 0:1])
                    nc.scalar.activation(z1[0:32, BL:2 * BL],
                                         cellps[0:32, BL:2 * BL],
                                         ACTF.Relu, bias=b1_sb[:, 1:2])
                    for g in range(3):
                        for cb in range(CB):
                            co = (g * CB + cb) * BL
                            wsl = slice(g * C + cb * 128,
                                        g * C + (cb + 1) * 128)
                            nc.tensor.matmul(
                                cellps[:, GI0 + co:GI0 + co + BL],
                                wih2t_sb[:, wsl], z1[:, 0:BL],
                                start=True, stop=True)
                            nc.tensor.matmul(
                                cellps[:, GH0 + co:GH0 + co + BL],
                                whh2t_sb[:, wsl], z1[:, BL:2 * BL],
                                start=True, stop=True)
                elif st == 2:  # cell 1 state update (DVE/ACT)
                    nc.vector.tensor_copy(gi_sb[:, :], cellps[:, GI0:GI1])
                    nc.vector.tensor_tensor(gates[:, :], gi_sb[:, :],
                                            cellps[:, GH0:GH1], ALU.add)
                    nc.scalar.activation(sgi[:, 0:NPLANE], gates[:, 0:NPLANE],
                                         ACTF.Sigmoid)
                    nc.scalar.activation(sgi[:, NPLANE:2 * NPLANE],
                                         gates[:, NPLANE:2 * NPLANE],
                                         ACTF.Sigmoid)
                    nc.scalar.activation(sgi[:, 2 * NPLANE:G3],
                                         gates[:, 2 * NPLANE:G3], ACTF.Tanh)
                    nc.vector.tensor_tensor(tmt[:, 0:NPLANE], sgi[:, 0:NPLANE],
                                            sgi[:, 2 * NPLANE:G3], ALU.mult)
                    if first_layer:
                        nc.vector.tensor_copy(ct[:, :], tmt[:, 0:NPLANE])
                    else:
                        nc.vector.tensor_tensor(
                            tmt[:, NPLANE:2 * NPLANE],
                            sgi[:, NPLANE:2 * NPLANE], ct[:, :], ALU.mult)
                        nc.vector.tensor_tensor(ct[:, :], tmt[:, 0:NPLANE],
                                                tmt[:, NPLANE:2 * NPLANE],
                                                ALU.add)
                    nc.scalar.activation(ht[:, :], ct[:, :], ACTF.Sigmoid)
                elif st == 3:  # cell 2 hh path from ht[0] (PE)
                    for cb in range(CB):
                        nc.tensor.matmul(
                            cellps[0:32, 2 * BL:2 * BL + 1],
                            whh1t_sb[:, cb * 32:(cb + 1) * 32],
                            ht[:, cb * BL:cb * BL + 1],
                            start=(cb == 0), stop=(cb == CB - 1))
                    nc.scalar.activation(z1[0:32, 2 * BL:2 * BL + 1],
                                         cellps[0:32, 2 * BL:2 * BL + 1],
                                         ACTF.Relu, bias=b1_sb[:, 1:2])
                    for g in range(3):
                        for cb in range(CB):
                            j = G20 + g * CB + cb
                            nc.tensor.matmul(
                                cellps[:, j:j + 1],
                                whh2t_sb[:, g * C + cb * 128:
                                         g * C + (cb + 1) * 128],
                                z1[:, 2 * BL:2 * BL + 1],
                                start=True, stop=True)
                else:  # st == 4: cell 2 -> s = 1 + out_h (DVE/ACT)
                    # gates2 = (g_i + bias) + g_h2 broadcast over batch
                    nc.vector.tensor_tensor(
                        gates[:, :].rearrange("p (j b) -> p j b", b=BL),
                        gi_sb[:, :].rearrange("p (j b) -> p j b", b=BL),
                        cellps[:, G20:G21].unsqueeze(-1).broadcast_to(
                            [128, 12, BL]),
                        ALU.add)
                    nc.scalar.activation(sgi[:, 0:NPLANE], gates[:, 0:NPLANE],
                                         ACTF.Sigmoid)
                    nc.scalar.activation(sgi[:, NPLANE:2 * NPLANE],
                                         gates[:, NPLANE:2 * NPLANE],
                                         ACTF.Sigmoid)
                    nc.scalar.activation(sgi[:, 2 * NPLANE:G3],
                                         gates[:, 2 * NPLANE:G3], ACTF.Tanh)
                    nc.vector.tensor_tensor(tmt[:, 0:NPLANE], sgi[:, 0:NPLANE],
                                            sgi[:, 2 * NPLANE:G3], ALU.mult)
                    # ncx2 = sig(f2)*ct[0] + sig(i2)*tanh(c2)
                    for cb in range(CB):
                        bs = cb * BL
                        nc.vector.scalar_tensor_tensor(
                            tmt[:, NPLANE + bs:NPLANE + bs + BL],
                            sgi[:, NPLANE + bs:NPLANE + bs + BL],
                            ct[:, bs:bs + 1], tmt[:, bs:bs + BL],
                            ALU.mult, ALU.add)
                    nc.scalar.activation(tmt[:, 2 * NPLANE:G3],
                                         tmt[:, NPLANE:2 * NPLANE],
                                         ACTF.Sigmoid)
                    nc.vector.tensor_scalar(s_sb[:, :], tmt[:, 2 * NPLANE:G3],
                                            1.0, None, ALU.add)

            def psview(ps):
                return ps[:, :].rearrange(
                    "p (h c) -> p h c", c=512)[:, :, 0:NCHUNK]

            def emit_conv(pl, taps, extra=None):
                """PE taps of one plane -> 2-bank PSUM tile (halves at 0/512).
                extra = (stationary AP, (dy, dx)) appended to the group."""
                cb = pl // BL
                ps = cvp.tile([128, 1024], F32, tag="cps", name="cps")
                n = len(taps) + (1 if extra is not None else 0)
                for hf in range(2):
                    r0 = hf * HALF
                    dst = ps[:, hf * 512:hf * 512 + NCHUNK]
                    for i, (dy, dx) in enumerate(taps):
                        t = ALL_TAPS.index((dy, dx))
                        nc.tensor.matmul(
                            dst,
                            diag_sb[:, (cb * 9 + t) * 128:
                                    (cb * 9 + t + 1) * 128],
                            tap(pl, dy, dx, r0, HALF),
                            start=(i == 0), stop=(i == n - 1))
                    if extra is not None:
                        st_ap, (dy, dx) = extra
                        nc.tensor.matmul(dst, st_ap, tap(pl, dy, dx, r0, HALF),
                                         start=False, stop=True)
                return ps

            def emit_plane_c(pl, d00, last):
                """Type C: all 9 taps on PE with s folded into the center
                diag; ACT evicts the finished y with pooled accumulation."""
                cb = pl // BL
                ps = emit_conv(pl, PE_TAPS8, extra=(d00[:, :], (0, 0)))
                if last:
                    ost = osp.tile([128, HW], F32, tag="ost", name="ost")
                    nc.scalar.activation(
                        ost[:, :].rearrange("p (h c) -> p h c", c=NCHUNK),
                        psview(ps), ACTF.Copy)
                    b = pl % BL
                    (nc.sync if pl % 2 == 0 else nc.scalar).dma_start(
                        y_d[b - 1, cb * 128:(cb + 1) * 128, :, :],
                        ost[:, :].rearrange("p (h w) -> p h w", w=W))
                else:
                    nc.scalar.activation(intr(pl), psview(ps), ACTF.Copy,
                                         accum_out=pooled[:, pl:pl + 1])

            def emit_chain(pl, ps):
                """Type B DVE work: spill via ACT, 3 dx=0 taps as TS+TT."""
                cb = pl // BL
                sc = spl.tile([128, HW], BF16, tag="spill", name="sc")
                nc.scalar.activation(
                    sc[:, :].rearrange("p (h c) -> p h c", c=NCHUNK),
                    psview(ps), ACTF.Copy)
                scv = sc[:, :].rearrange("p (r w) -> p r w", w=W)
                acc = acp.tile([128, HW], BF16, tag="acc", name="acc")
                accv = acc[:, :].rearrange("p (r w) -> p r w", w=W)
                prev = scv
                for k, (dy, _) in enumerate(DVE_TAPS):
                    tp = tpp.tile([128, HW], BF16, tag="tp", name="tp")
                    tpv = tp[:, :].rearrange("p (r w) -> p r w", w=W)
                    nc.vector.tensor_scalar(
                        tpv, tap(pl, dy, 0),
                        dwv_sb[:, cb * 2 + k:cb * 2 + k + 1], None, ALU.mult)
                    nc.vector.tensor_tensor(accv, tpv, prev, ALU.add)
                    prev = accv
                return accv

            def emit_final_b(pl, accv, last):
                cb = pl // BL
                if last:
                    ost = osp.tile([128, HW], F32, tag="ost", name="ost")
                    nc.vector.scalar_tensor_tensor(
                        ost[:, :].rearrange("p (r w) -> p r w", w=W),
                        intr(pl), w00s[:, pl:pl + 1], accv,
                        ALU.mult, ALU.add)
                    b = pl % BL
                    (nc.sync if pl % 2 == 0 else nc.scalar).dma_start(
                        y_d[b - 1, cb * 128:(cb + 1) * 128, :, :],
                        ost[:, :].rearrange("p (h w) -> p h w", w=W))
                else:
                    nc.vector.scalar_tensor_tensor(
                        intr(pl), intr(pl), w00s[:, pl:pl + 1], accv,
                        ALU.mult, ALU.add,
                        accum_out=pooled[:, pl:pl + 1])

            def emit_plane_b(pl, last):
                ps = emit_conv(pl, PE_TAPS6)
                accv = emit_chain(pl, ps)
                emit_final_b(pl, accv, last)

            def emit_input():
                # DRAM -> stage (paired planes, both HWDGE rings) -> padded
                # bf16 layout; the conversion op also accumulates the
                # layer-1 pooled sums.
                for cb in range(CB):
                    groups = [(0, 2), (2, 2), (4, 2), (6, 2), (8, 1)]
                    for gi, (b0, nb) in enumerate(groups):
                        stage = sp.tile([128, 2 * HW], F32, tag="stage",
                                        name="stage")
                        eng = nc.sync if gi % 2 == 0 else nc.scalar
                        eng.dma_start(
                            stage[:, 0:nb * HW].rearrange(
                                "p (b hw) -> p b hw", hw=HW),
                            x_d[b0:b0 + nb, cb * 128:(cb + 1) * 128, :, :]
                            .rearrange("b c h w -> c b (h w)"))
                        for k in range(nb):
                            pl = cb * BL + b0 + k
                            seg = stage[:, k * HW:(k + 1) * HW].rearrange(
                                "p (r w) -> p r w", w=W)
                            if pl % 2 == 0:
                                nc.scalar.activation(
                                    intr(pl), seg, ACTF.Copy,
                                    accum_out=pooled[:, pl:pl + 1])
                            else:
                                nc.vector.tensor_scalar(
                                    intr(pl), seg, 1.0, 0.0, ALU.mult,
                                    ALU.add, accum_out=pooled[:, pl:pl + 1])

            if skip_io:
                emit_input()
            if skip_cells:
                nc.vector.memset(s_sb[:, :], 1.5)
                nc.vector.tensor_tensor(w00s[:, :], s_sb[:, :],
                                        dw00r_sb[:, :], ALU.add)

            def emit_body():
                if not skip_io:
                    emit_input()
                for layer in range(num_layers):
                    last = layer == num_layers - 1
                    planes = [cb * BL + b for cb in range(CB)
                              for b in range(BL)
                              if not (last and b == 0)]
                    # the last n_c planes of the layer are type C (their PE
                    # groups need s via the folded diag, so they go late);
                    # everything before them is type B
                    nb_planes = planes[:len(planes) - n_c]
                    nc_planes = planes[len(planes) - n_c:]

                    def emit_d00s():
                        nc.vector.tensor_tensor(w00s[:, :], s_sb[:, :],
                                                dw00r_sb[:, :], ALU.add)
                        d00s = []
                        for pl in nc_planes:
                            d00 = d0p.tile([128, 128], BF16, tag="d00",
                                           name="d00")
                            nc.vector.tensor_scalar(
                                d00[:, :], ident_sb[:, :],
                                w00s[:, pl:pl + 1], None, ALU.mult)
                            d00s.append(d00)
                        return d00s

                    if skip_cells:
                        d00s = emit_d00s()
                        for pl in nb_planes:
                            emit_plane_b(pl, last)
                        for i, pl in enumerate(nc_planes):
                            emit_plane_c(pl, d00s[i], last)
                        continue
                    # Phase 1: all type-B tap work (PE convs, ACT evicts,
                    # DVE chains) with the cell chain interleaved after the
                    # first couple of planes.  Finals are held: they need s.
                    held = []
                    d00s = None
                    # layer 0's pooled only completes once the whole input
                    # has streamed in, so its cell chain goes near the end
                    # of phase 1; later layers interleave it right away
                    i0 = 2 if layer > 0 else max(2, len(nb_planes) - 7)
                    for i, pl in enumerate(nb_planes):
                        ps = emit_conv(pl, PE_TAPS6)
                        held.append((pl, emit_chain(pl, ps)))
                        if i0 <= i <= i0 + 4:
                            cell_stage(i - i0, layer == 0)
                            if i == i0 + 4:
                                d00s = emit_d00s()
                    if d00s is None:  # tiny plane counts
                        for st in range(5):
                            cell_stage(st, layer == 0)
                        d00s = emit_d00s()
                    # Phase 2: s-gated work - B finals interleaved with the
                    # type-C planes so PE/ACT keep working through the tail
                    ic = 0
                    for j, (pl, accv) in enumerate(held):
                        emit_final_b(pl, accv, last)
                        if (ic < len(nc_planes)
                                and (j + 1) * len(nc_planes) >= (ic + 1) * len(held)):
                            emit_plane_c(nc_planes[ic], d00s[ic], last)
                            ic += 1
                    while ic < len(nc_planes):
                        emit_plane_c(nc_planes[ic], d00s[ic], last)
                        ic += 1

            if iters == 1:
                emit_body()
            else:
                with tc.For_i(0, iters, 1, staggered_reset=True):
                    emit_body()

    nc.compile()
    return nc


def prep_inputs(x, w_ih_l1, b_ih_l1, w_ih_l2, b_ih_l2,
                w_hh_l1, b_hh_l1, w_hh_l2, b_hh_l2, dw_kernel):
    """Host-side prep: per-core input maps (weights replicated)."""
    x = np.ascontiguousarray(np.asarray(x, dtype=np.float32))
    dw = np.asarray(dw_kernel, np.float32).reshape(C, 3, 3)
    diag = np.zeros((CB, 9, 128, 128), np.float32)
    idx = np.arange(128)
    for cb in range(CB):
        for t, (dy, dx) in enumerate(ALL_TAPS):
            diag[cb, t, idx, idx] = dw[cb * 128:(cb + 1) * 128, dy + 1, dx + 1]
    dwv = np.zeros((128, CB * 2), np.float32)
    for cb in range(CB):
        for k, (dy, _) in enumerate(DVE_TAPS):
            dwv[:, cb * 2 + k] = dw[cb * 128:(cb + 1) * 128, dy + 1, 1]
    # center-tap weight replicated per plane (for the s-folded diag build)
    dw00r = np.zeros((128, NPLANE), np.float32)
    for cb in range(CB):
        for b in range(BL):
            dw00r[:, cb * BL + b] = dw[cb * 128:(cb + 1) * 128, 1, 1]
    # l2 weights pre-transposed with the summed gate bias as an extra row
    # (pairs with the ones-row of z1)
    wih2t = np.concatenate(
        [np.asarray(w_ih_l2, np.float32).T,
         (np.asarray(b_ih_l2, np.float32)
          + np.asarray(b_hh_l2, np.float32))[None, :]], axis=0)
    whh2t = np.concatenate(
        [np.asarray(w_hh_l2, np.float32).T,
         np.zeros((1, 3 * C), np.float32)], axis=0)
    common = {
        "diag": diag.reshape(CB * 9 * 128, 128).astype(ml_dtypes.bfloat16),
        "ident": np.eye(128, dtype=ml_dtypes.bfloat16),
        "dwv": np.ascontiguousarray(dwv),
        "dw00r": np.ascontiguousarray(dw00r),
        "wih1t": np.ascontiguousarray(
            (np.asarray(w_ih_l1, np.float32) / HW).T),
        "whh1t": np.ascontiguousarray(np.asarray(w_hh_l1, np.float32).T),
        "wih2t": np.ascontiguousarray(wih2t),
        "whh2t": np.ascontiguousarray(whh2t),
        "b1": np.ascontiguousarray(np.stack(
            [np.asarray(b_ih_l1, np.float32),
             np.asarray(b_hh_l1, np.float32)], axis=1)),
    }
    return [dict(common, x=np.ascontiguousarray(np.concatenate(
        [x[0:1], x[i * B_SH:(i + 1) * B_SH]], axis=0)))
        for i in range(N_CORES)]


_cache = {}


def kernel(**inputs) -> np.ndarray:
    num_layers = int(inputs["num_layers"])
    if num_layers not in _cache:
        _cache[num_layers] = build_program(num_layers=num_layers, iters=1, n_c=6)
    nc = _cache[num_layers]
    in_maps = prep_inputs(
        inputs["x"], inputs["w_ih_l1"], inputs["b_ih_l1"], inputs["w_ih_l2"],
        inputs["b_ih_l2"], inputs["w_hh_l1"], inputs["b_hh_l1"],
        inputs["w_hh_l2"], inputs["b_hh_l2"], inputs["dw_kernel"])
    res = run_bass_kernel_spmd(nc, in_maps, list(range(N_CORES)))
    return np.concatenate([res.results[i]["y"] for i in range(N_CORES)],
                          axis=0).astype(np.float32)
